# revision 69
# baseline (speedup 1.0000x reference)
"""Trainium2 Bass kernel for EngramMemory embedding lookup.

Computation (matches the jax reference):
  tokens [4, 4096] int64, tables [8, 2048, 256] f32
  For each order o in (2, 3) and head h in 0..3, hash the o-gram ending at
  each position into a bucket (mod 2048) and gather the 256-wide embedding
  row; concatenate all 8 (order, head) chunks -> out [4, 4096, 2048] f32.

The hash recurrence  h = (h*P + tok + seed) % 2048  telescopes to an affine
form mod 2048 (P = 1000003 == 579, P^2 == 1417 mod 2048):
  order2 bucket_h = (x2 + C2[h]) & 2047,  x2 = 579*t[s-1] + t[s]
  order3 bucket_h = (x3 + C3[h]) & 2047,  x3 = 1417*t[s-2] + 579*t[s-1] + t[s]
(tokens pre-masked to 11 bits so products stay < 2^24, exact in the DVE's
fp32 multiply path; C_oh = seed_oh * (P+..+1) mod 2048).

Within an order, the four heads differ only by the additive constant C_oh.
Host-side we therefore build per-order fused tables
  F_o[j] = concat_h T_oh[(j + C_oh) & 2047]        -> [2048, 4*256] f32
so the device gathers ONE 4 KiB row per (position, order) with the shared
index j = x_o & 2047. This cuts SWDGE gather descriptors 4x (the Q7
descriptor-generation rate, ~8.5 ns/descriptor on one core pair, is the
serial bottleneck) and makes every gathered row land contiguously in the
output layout.

Sharding: data-parallel over the 16384 token positions, 2048 per core
(core c takes batch row c//2, half c%2); tables replicated. Per core:
  1. DMA shifted token streams [128, 3*128] i32 (host supplies them in the
     index-tile layout: cell (p, m) of block k is the token at position
     16m + p%16 - (2-k), replicated across the 8 partition groups).
  2. 17 DVE int ops -> index tile [128, 256] i16: cell (p, o*128 + m) =
     fused-row index for order o at position 16m + p%16. This matches the
     SWDGE gather index wrap (idx j at partition j%16, col j//16, i.e.
     j = position), is born replicated for all Q7 pairs, and makes the
     gathered rows land position-major.
  3. 16x dma_gather, 8 position-range chunks per order (one call tops out
     between 1024 and 2048 descriptors = the 128-entry/ring in-flight cap x
     16 rings; chunks also pipeline with the writeback; a 128-position
     first chunk rides the fast-path index slice), queues cycling over the
     4 Q7 core pairs.
  4. 16 HWDGE out-DMAs (3-dim APs spanning all 128 partitions -> all 16 DMA
     engines), alternating between the sync and scalar HWDGE engines since
     one engine's descriptor generation (~2.5 us/MiB) cannot feed the DMA
     bus alone.

Measured on TRN2 (8 cores): ~103 us HW exec (best; ambient shared-HBM
contention adds up to ~20% in bad phases), vs a ~89 us saturated-DMA-bus
floor (32 MiB/core: 16 MiB random 4 KiB gather reads + 16 MiB writeback at
~25 GB/s x 16 engines; the DMA engines measure 100% busy in the steady
phase). The ~19.5 us serial prefix is preamble + token load + hash + first
descriptor generation and ucode IRAM fetch.
"""

import os
import sys
from contextlib import ExitStack

import numpy as np

for _p in ("/opt/trn_rl_repo", os.path.expanduser("~/.axon_site/_ro/trn_rl_repo")):
    if os.path.isdir(_p) and _p not in sys.path:
        sys.path.insert(0, _p)

import concourse.bacc as bacc
import concourse.mybir as mybir

N_CORES = 8
B, S, D = 4, 4096, 2048
POS = (B * S) // N_CORES  # 2048 positions per core
NUM_BUCKETS = 2048
FUSED_DIM = 4 * 256  # one fused row: 4 heads x head_dim
N_ORDERS = 2

# Affine hash constants (see module docstring). seeds = 1337 + 97*o + 17*h.
_C2 = [((1337 + 97 * 2 + 17 * h) * 580) % 2048 for h in range(4)]
_C3 = [((1337 + 97 * 3 + 17 * h) * 1997) % 2048 for h in range(4)]

_AND = mybir.AluOpType.bitwise_and
_MUL = mybir.AluOpType.mult
_ADD = mybir.AluOpType.add

# gather chunk sizes per order, in 128-position slots (sum 16 = 2048
# positions). The tiny first chunk rides a fast-path index slice computed
# right after the first token block lands, pulling the first descriptor
# generation (and the gather ucode's IRAM fetch) several us earlier. queue
# (= Q7 core pair) cycles per call so desc-gen rings don't serialize.
CHUNK_SLOTS = [1, 2, 2, 2, 2, 2, 2, 3]
N_SPLIT = len(CHUNK_SLOTS)
GATHER_QUEUES = [i % 4 for i in range(2 * N_SPLIT)]


def _build_nc():
    from concourse.library_config import mlp

    nc = bacc.Bacc("TRN2", num_swdge_queues=max(GATHER_QUEUES) + 1)
    # tokens[p, k*128 + m] = token at position 16m + p%16 - (2-k)  (0-padded),
    # i.e. col-block k holds the (2-k)-shifted token stream in the layout the
    # index tile needs (idx j for position 16m+q lives at partition q, col m)
    tok = nc.declare_dram_parameter("tokens", [128, 384], mybir.dt.int32, isOutput=False)
    tbl = nc.declare_dram_parameter(
        "tables", [N_ORDERS * NUM_BUCKETS, FUSED_DIM], mybir.dt.float32, isOutput=False
    )
    out = nc.declare_dram_parameter("out", [POS, D], mybir.dt.float32, isOutput=True)

    with (
        nc.Block() as block,
        ExitStack() as stack,
    ):
        sem = lambda name: stack.enter_context(nc.semaphore(name))  # noqa: E731
        sb = lambda name, shape, dt: stack.enter_context(nc.sbuf_tensor(name, shape, dt))  # noqa: E731

        s_load = [sem(f"s_load{k}") for k in range(3)]
        v_hash = sem("v_hash")
        s_g = [sem(f"s_g{i}") for i in range(2 * N_SPLIT)]  # one per gather call (per queue)
        s_out = sem("s_out")
        s_out_b = sem("s_out_b")

        # All hash tiles live on all 128 partitions: the hash is computed
        # 8x redundantly (DVE cost is per-partition elements, so this is
        # free) and the index tile comes out already replicated for the
        # Q7 pairs -- no replication DMAs on the critical path.
        v64 = sb("v64", [128, 384], mybir.dt.int32)
        t0 = sb("t0", [128, 128], mybir.dt.int32)
        t1 = sb("t1", [128, 128], mybir.dt.int32)
        t2 = sb("t2", [128, 128], mybir.dt.int32)
        t1m = sb("t1m", [128, 128], mybir.dt.int32)
        t2m = sb("t2m", [128, 128], mybir.dt.int32)
        s2 = sb("s2", [128, 128], mybir.dt.int32)
        s3 = sb("s3", [128, 128], mybir.dt.int32)
        j2 = sb("j2", [128, 128], mybir.dt.int32)
        j3 = sb("j3", [128, 128], mybir.dt.int32)
        idx = sb("idx", [128, N_ORDERS * 128], mybir.dt.int16)
        dst = [sb(f"dst{o}", [128, 16, FUSED_DIM], mybir.dt.float32) for o in range(N_ORDERS)]

        @block.sync
        def _(sync):
            # three col-block loads, ordered so the order-2 hash operands
            # (t[s-1], t[s]) arrive first and the hash can start sooner
            sync.dma_start(v64[:, 128:256], tok[:, 128:256]).then_inc(s_load[0], 16)
            sync.dma_start(v64[:, 256:384], tok[:, 256:384]).then_inc(s_load[1], 16)
            sync.dma_start(v64[:, 0:128], tok[:, 0:128]).then_inc(s_load[2], 16)
            # Gather rows land pos-major: row for position 128*sl + p sits at
            # (partition p, slot sl). Each out-DMA is a clean 3-dim AP over
            # all 128 partitions -> all 16 DMA engines. HWDGE descriptor
            # generation runs ~2.5us/MiB per engine -- barely above the DMA
            # bus rate -- so the writeback DMAs alternate between the two
            # HWDGE engines (sync here, scalar below) to keep the bus fed.
            ov = out.rearrange("(sl p) (o e) -> o p sl e", sl=16, p=128, e=FUSED_DIM)
            n_sync = 0
            for o in range(N_ORDERS):
                sl0 = 0
                for h, spc in enumerate(CHUNK_SLOTS):
                    if (N_SPLIT * o + h) % 2 == 0:
                        sync.wait_ge(s_g[N_SPLIT * o + h], 16)
                        sync.dma_start(
                            ov[o][:, sl0 : sl0 + spc, :],
                            dst[o][:, sl0 : sl0 + spc, :],
                        ).then_inc(s_out, 16)
                        n_sync += 1
                    sl0 += spc
            sync.wait_ge(s_out, 16 * n_sync)
            sync.wait_ge(s_out_b, 16 * (N_SPLIT * N_ORDERS - n_sync))

        @block.scalar
        def _(sc):
            ovb = out.rearrange("(sl p) (o e) -> o p sl e", sl=16, p=128, e=FUSED_DIM)
            for o in range(N_ORDERS):
                sl0 = 0
                for h, spc in enumerate(CHUNK_SLOTS):
                    if (N_SPLIT * o + h) % 2 == 1:
                        sc.wait_ge(s_g[N_SPLIT * o + h], 16)
                        sc.dma_start(
                            ovb[o][:, sl0 : sl0 + spc, :],
                            dst[o][:, sl0 : sl0 + spc, :],
                        ).then_inc(s_out_b, 16)
                    sl0 += spc

        @block.vector
        def _(v):
            # Every DVE op bumps v_hash; DVE has no same-engine interlocks, so
            # dependent ops wait on the cumulative count of their producers.
            # Order-2 path first: its index tile gates the first gathers.
            n = 0

            def op(wait, fn, *args):
                nonlocal n
                if wait:
                    v.wait_ge(v_hash, wait)
                fn(*args).then_inc(v_hash, 1)
                n += 1

            # fast path: idx cols 0:8 (gather chunk 0, 128 positions) on
            # column slices, so the first desc-gen starts ~4us earlier
            v.wait_ge(s_load[0], 16)
            v.wait_ge(s_load[1], 16)
            op(0, v.tensor_scalar, t1[:, 0:8], v64[:, 128:136], 2047, None, _AND)  # 1
            op(0, v.tensor_scalar, t0[:, 0:8], v64[:, 256:264], 2047, None, _AND)  # 2
            op(1, v.tensor_scalar, t1m[:, 0:8], t1[:, 0:8], 579, None, _MUL)       # 3
            op(3, v.tensor_tensor, s2[:, 0:8], t1m[:, 0:8], t0[:, 0:8], _ADD)      # 4
            op(4, v.tensor_scalar, j2[:, 0:8], s2[:, 0:8], 2047, None, _AND)       # 5
            op(5, v.tensor_copy, idx[:, 0:8], j2[:, 0:8])                          # 6
            # remainder of order-2 (cols 8:128)
            op(0, v.tensor_scalar, t1[:, 8:128], v64[:, 136:256], 2047, None, _AND)  # 7
            op(0, v.tensor_scalar, t0[:, 8:128], v64[:, 264:384], 2047, None, _AND)  # 8
            op(7, v.tensor_scalar, t1m[:, 8:128], t1[:, 8:128], 579, None, _MUL)     # 9
            op(9, v.tensor_tensor, s2[:, 8:128], t1m[:, 8:128], t0[:, 8:128], _ADD)  # 10
            op(10, v.tensor_scalar, j2[:, 8:128], s2[:, 8:128], 2047, None, _AND)    # 11
            op(11, v.tensor_copy, idx[:, 8:128], j2[:, 8:128])                       # 12
            # order-3 (full width; s2 complete after op 10)
            v.wait_ge(s_load[2], 16)
            op(0, v.tensor_scalar, t2[:, :], v64[:, 0:128], 2047, None, _AND)     # 13
            op(13, v.tensor_scalar, t2m[:, :], t2[:, :], 1417, None, _MUL)        # 14
            op(14, v.tensor_tensor, s3[:, :], t2m[:, :], s2[:, :], _ADD)          # 15
            op(15, v.tensor_scalar, j3[:, :], s3[:, :], 2047, None, _AND)         # 16
            op(16, v.tensor_copy, idx[:, 128:256], j3[:, :])                      # 17
            assert n == 17

        @block.gpsimd
        def _(gp):
            gp.load_library(mlp)
            # Pool-sequencer instruction dispatch costs ~0.4us apiece, so:
            # (a) stage the num_idxs registers ONCE, before the data-dependent
            # wait (to_reg per call emitted 16 MOVEs, ~5us of them on the
            # critical path before the first gather); (b) emit only the 3
            # distinct v_hash waits instead of one per call.
            nregs = {}
            for spc in sorted(set(CHUNK_SLOTS)):
                r = stack.enter_context(gp.register(f"nidx{spc}"))
                gp.reg_mov(r, 128 * spc)
                nregs[spc] = gp.snap(r)
            # one dma_gather tops out between 1024 and 2048 descriptors on HW
            # (SWDGE ring in-flight cap); finer chunks also start transfers
            # earlier and interleave the writeback DMAs with gather reads
            last_wait = 0
            for o in range(N_ORDERS):
                sl0 = 0
                for h, spc in enumerate(CHUNK_SLOTS):
                    # chunk-0 fast path ready at 6, rest of order-2 at 12,
                    # order-3 at 17
                    need = (6 if h == 0 else 12) if o == 0 else 17
                    if need > last_wait:
                        gp.wait_ge(v_hash, need)
                        last_wait = need
                    gp.dma_gather(
                        dst[o][:, sl0 : sl0 + spc, :],
                        tbl[NUM_BUCKETS * o : NUM_BUCKETS * (o + 1), :],
                        idx[:, 128 * o + 8 * sl0 : 128 * o + 8 * (sl0 + spc)],
                        128 * spc,
                        nregs[spc],
                        FUSED_DIM,
                        queue_num=GATHER_QUEUES[N_SPLIT * o + h],
                    ).then_inc(s_g[N_SPLIT * o + h], 16)
                    sl0 += spc

    nc.compile()
    return nc


_NC = None


def _get_nc():
    global _NC
    if _NC is None:
        _NC = _build_nc()
    return _NC


def _shard_tokens(tokens):
    """Per-core [128, 384] i32 tiles: col-block k in {0,1,2} holds the
    (2-k)-shifted token stream laid out so cell (p, m) is the token at
    position 16m + p%16 - (2-k) (0-padded past the row start), replicated
    across the 8 partition groups."""
    tokens = np.asarray(tokens, dtype=np.int64)
    maps = []
    q = np.arange(16)[:, None]  # partition % 16
    m = np.arange(128)[None, :]
    for c in range(N_CORES):
        b, s0 = c // 2, (c % 2) * POS
        halo = np.concatenate([np.zeros(2, np.int64), tokens[b]])  # halo[i] = t[i-2]
        blocks = [halo[s0 + 16 * m + q + k] for k in range(3)]  # [16, 128] each
        tile = np.concatenate(blocks, axis=1).astype(np.int32)  # [16, 384]
        maps.append(np.ascontiguousarray(np.tile(tile, (8, 1))))
    return maps


def _fuse_tables(tables):
    """[8, 2048, 256] -> [2*2048, 1024]: per-order head-concat with each head's
    table rotated by its additive hash constant, so all heads share one index."""
    tables = np.asarray(tables, dtype=np.float32).reshape(8, NUM_BUCKETS, 256)
    j = np.arange(NUM_BUCKETS)
    fused = np.empty((N_ORDERS, NUM_BUCKETS, 4, 256), np.float32)
    for h in range(4):
        fused[0, :, h, :] = tables[h][(j + _C2[h]) & (NUM_BUCKETS - 1)]
        fused[1, :, h, :] = tables[4 + h][(j + _C3[h]) & (NUM_BUCKETS - 1)]
    return np.ascontiguousarray(fused.reshape(N_ORDERS * NUM_BUCKETS, FUSED_DIM))


def _in_maps(tokens, tables):
    tbl = _fuse_tables(tables)
    return [{"tokens": tw, "tables": tbl} for tw in _shard_tokens(tokens)]


def kernel(tokens, tables):
    from concourse.bass_utils import run_bass_kernel_spmd

    res = run_bass_kernel_spmd(
        _get_nc(), _in_maps(tokens, tables), list(range(N_CORES))
    )
    parts = [np.asarray(res.results[c]["out"]) for c in range(N_CORES)]
    return np.concatenate(parts, axis=0).reshape(B, S, D)


def run_traced(tokens, tables, **kw):
    """Timing/profiling run; returns the full BassKernelResults."""
    from concourse.bass_utils import run_bass_kernel_spmd

    return run_bass_kernel_spmd(
        _get_nc(), _in_maps(tokens, tables), list(range(N_CORES)), trace=True, **kw
    )



# revision 75
# speedup vs baseline: 1.1251x; 1.1251x over previous
"""Trainium2 Bass kernel for EngramMemory embedding lookup.

Computation (matches the jax reference):
  tokens [4, 4096] int64, tables [8, 2048, 256] f32
  For each order o in (2, 3) and head h in 0..3, hash the o-gram ending at
  each position into a bucket (mod 2048) and gather the 256-wide embedding
  row; concatenate all 8 (order, head) chunks -> out [4, 4096, 2048] f32.

The hash recurrence  h = (h*P + tok + seed) % 2048  telescopes to an affine
form mod 2048 (P = 1000003 == 579, P^2 == 1417 mod 2048):
  order2 bucket_h = (x2 + C2[h]) & 2047,  x2 = 579*t[s-1] + t[s]
  order3 bucket_h = (x3 + C3[h]) & 2047,  x3 = 1417*t[s-2] + 579*t[s-1] + t[s]
(tokens pre-masked to 11 bits so products stay < 2^24, exact in the DVE's
fp32 multiply path; C_oh = seed_oh * (P+..+1) mod 2048).

Within an order, the four heads differ only by the additive constant C_oh.
Host-side we therefore build per-order fused tables
  F_o[j] = concat_h T_oh[(j + C_oh) & 2047]        -> [2048, 4*256] f32
so the device gathers ONE 4 KiB row per (position, order) with the shared
index j = x_o & 2047. This cuts SWDGE gather descriptors 4x (the Q7
descriptor-generation rate, ~8.5 ns/descriptor on one core pair, is the
serial bottleneck) and makes every gathered row land contiguously in the
output layout.

Sharding: data-parallel over the 16384 token positions, 2048 per core
(core c takes batch row c//2, half c%2); tables replicated. Per core:
  1. DMA shifted token streams [128, 3*128] i32 (host supplies them in the
     index-tile layout: cell (p, m) of block k is the token at position
     16m + p%16 - (2-k), replicated across the 8 partition groups).
  2. 17 DVE int ops -> index tile [128, 256] i16: cell (p, o*128 + m) =
     fused-row index for order o at position 16m + p%16. This matches the
     SWDGE gather index wrap (idx j at partition j%16, col j//16, i.e.
     j = position), is born replicated for all Q7 pairs, and makes the
     gathered rows land position-major.
  3. 16x dma_gather, 8 position-range chunks per order (one call tops out
     between 1024 and 2048 descriptors = the 128-entry/ring in-flight cap x
     16 rings; chunks also pipeline with the writeback; a 128-position
     first chunk rides the fast-path index slice), queues cycling over the
     4 Q7 core pairs.
  4. 16 HWDGE out-DMAs (3-dim APs spanning all 128 partitions -> all 16 DMA
     engines), alternating between the sync and scalar HWDGE engines since
     one engine's descriptor generation (~2.5 us/MiB) cannot feed the DMA
     bus alone.

Measured on TRN2 (8 cores): ~103 us HW exec (best; ambient shared-HBM
contention adds up to ~20% in bad phases), vs a ~89 us saturated-DMA-bus
floor (32 MiB/core: 16 MiB random 4 KiB gather reads + 16 MiB writeback at
~25 GB/s x 16 engines; the DMA engines measure 100% busy in the steady
phase). The ~19.5 us serial prefix is preamble + token load + hash + first
descriptor generation and ucode IRAM fetch.
"""

import os
import sys
from contextlib import ExitStack

import numpy as np

for _p in ("/opt/trn_rl_repo", os.path.expanduser("~/.axon_site/_ro/trn_rl_repo")):
    if os.path.isdir(_p) and _p not in sys.path:
        sys.path.insert(0, _p)

import concourse.bacc as bacc
import concourse.mybir as mybir

N_CORES = 8
B, S, D = 4, 4096, 2048
POS = (B * S) // N_CORES  # 2048 positions per core
NUM_BUCKETS = 2048
FUSED_DIM = 4 * 256  # one fused row: 4 heads x head_dim
N_ORDERS = 2

# Affine hash constants (see module docstring). seeds = 1337 + 97*o + 17*h.
_C2 = [((1337 + 97 * 2 + 17 * h) * 580) % 2048 for h in range(4)]
_C3 = [((1337 + 97 * 3 + 17 * h) * 1997) % 2048 for h in range(4)]

_AND = mybir.AluOpType.bitwise_and
_MUL = mybir.AluOpType.mult
_ADD = mybir.AluOpType.add

# gather chunk sizes per order, in 128-position slots (sum 16 = 2048
# positions). The tiny first chunk rides a fast-path index slice computed
# right after the first token block lands, pulling the first descriptor
# generation (and the gather ucode's IRAM fetch) several us earlier. queue
# (= Q7 core pair) cycles per call so desc-gen rings don't serialize.
CHUNK_SLOTS = [1, 2, 2, 2, 2, 2, 2, 3]
N_SPLIT = len(CHUNK_SLOTS)
GATHER_QUEUES = [i % 4 for i in range(2 * N_SPLIT)]


def _build_nc():
    from concourse.library_config import mlp

    nc = bacc.Bacc("TRN2", num_swdge_queues=max(GATHER_QUEUES) + 1)
    # tokens[p, k*128 + m] = token at position 16m + p%16 - (2-k)  (0-padded),
    # i.e. col-block k holds the (2-k)-shifted token stream in the layout the
    # index tile needs (idx j for position 16m+q lives at partition q, col m)
    # cols 384:400 duplicate the fast-path operand slices (cols 128:136 and
    # 256:264) so one 8 KiB DMA + one completion sem gates the first hash ops
    tok = nc.declare_dram_parameter("tokens", [128, 400], mybir.dt.int32, isOutput=False)
    tbl = nc.declare_dram_parameter(
        "tables", [N_ORDERS * NUM_BUCKETS, FUSED_DIM], mybir.dt.float32, isOutput=False
    )
    out = nc.declare_dram_parameter("out", [POS, D], mybir.dt.float32, isOutput=True)

    with (
        nc.Block() as block,
        ExitStack() as stack,
    ):
        sem = lambda name: stack.enter_context(nc.semaphore(name))  # noqa: E731
        sb = lambda name, shape, dt: stack.enter_context(nc.sbuf_tensor(name, shape, dt))  # noqa: E731

        s_load = [sem(f"s_load{k}") for k in range(3)]
        s_mini = sem("s_mini")
        v_hash = sem("v_hash")
        s_g = [sem(f"s_g{i}") for i in range(2 * N_SPLIT)]  # one per gather call (per queue)
        s_out = sem("s_out")
        s_out_b = sem("s_out_b")

        # All hash tiles live on all 128 partitions: the hash is computed
        # 8x redundantly (DVE cost is per-partition elements, so this is
        # free) and the index tile comes out already replicated for the
        # Q7 pairs -- no replication DMAs on the critical path.
        v64 = sb("v64", [128, 384], mybir.dt.int32)
        v64x = sb("v64x", [128, 16], mybir.dt.int32)
        t0 = sb("t0", [128, 128], mybir.dt.int32)
        t1 = sb("t1", [128, 128], mybir.dt.int32)
        t2 = sb("t2", [128, 128], mybir.dt.int32)
        t1m = sb("t1m", [128, 128], mybir.dt.int32)
        t2m = sb("t2m", [128, 128], mybir.dt.int32)
        s2 = sb("s2", [128, 128], mybir.dt.int32)
        s3 = sb("s3", [128, 128], mybir.dt.int32)
        j2 = sb("j2", [128, 128], mybir.dt.int32)
        j3 = sb("j3", [128, 128], mybir.dt.int32)
        idx = sb("idx", [128, N_ORDERS * 128], mybir.dt.int16)
        dst = [sb(f"dst{o}", [128, 16, FUSED_DIM], mybir.dt.float32) for o in range(N_ORDERS)]

        @block.sync
        def _(sync):
            # fast-path mini-block first, then the three col-block loads
            # ordered so the order-2 hash operands arrive before order-3's
            sync.dma_start(v64x[:, :], tok[:, 384:400]).then_inc(s_mini, 16)
            sync.dma_start(v64[:, 128:256], tok[:, 128:256]).then_inc(s_load[0], 16)
            sync.dma_start(v64[:, 256:384], tok[:, 256:384]).then_inc(s_load[1], 16)
            sync.dma_start(v64[:, 0:128], tok[:, 0:128]).then_inc(s_load[2], 16)
            # Gather rows land pos-major: row for position 128*sl + p sits at
            # (partition p, slot sl). Each out-DMA is a clean 3-dim AP over
            # all 128 partitions -> all 16 DMA engines. HWDGE descriptor
            # generation runs ~2.5us/MiB per engine -- barely above the DMA
            # bus rate -- so the writeback DMAs alternate between the two
            # HWDGE engines (sync here, scalar below) to keep the bus fed.
            ov = out.rearrange("(sl p) (o e) -> o p sl e", sl=16, p=128, e=FUSED_DIM)
            n_sync = 0
            for o in range(N_ORDERS):
                sl0 = 0
                for h, spc in enumerate(CHUNK_SLOTS):
                    if (N_SPLIT * o + h) % 2 == 0:
                        sync.wait_ge(s_g[N_SPLIT * o + h], 16)
                        sync.dma_start(
                            ov[o][:, sl0 : sl0 + spc, :],
                            dst[o][:, sl0 : sl0 + spc, :],
                        ).then_inc(s_out, 16)
                        n_sync += 1
                    sl0 += spc
            sync.wait_ge(s_out, 16 * n_sync)
            sync.wait_ge(s_out_b, 16 * (N_SPLIT * N_ORDERS - n_sync))

        @block.scalar
        def _(sc):
            ovb = out.rearrange("(sl p) (o e) -> o p sl e", sl=16, p=128, e=FUSED_DIM)
            for o in range(N_ORDERS):
                sl0 = 0
                for h, spc in enumerate(CHUNK_SLOTS):
                    if (N_SPLIT * o + h) % 2 == 1:
                        sc.wait_ge(s_g[N_SPLIT * o + h], 16)
                        sc.dma_start(
                            ovb[o][:, sl0 : sl0 + spc, :],
                            dst[o][:, sl0 : sl0 + spc, :],
                        ).then_inc(s_out_b, 16)
                    sl0 += spc

        @block.vector
        def _(v):
            # Every DVE op bumps v_hash; DVE has no same-engine interlocks, so
            # dependent ops wait on the cumulative count of their producers.
            # Order-2 path first: its index tile gates the first gathers.
            n = 0

            def op(wait, fn, *args):
                nonlocal n
                if wait:
                    v.wait_ge(v_hash, wait)
                fn(*args).then_inc(v_hash, 1)
                n += 1

            # fast path: idx cols 0:8 (gather chunk 0, 128 positions) from the
            # mini-block, so the first desc-gen starts several us earlier
            v.wait_ge(s_mini, 16)
            op(0, v.tensor_scalar, t1[:, 0:8], v64x[:, 0:8], 2047, None, _AND)     # 1
            op(0, v.tensor_scalar, t0[:, 0:8], v64x[:, 8:16], 2047, None, _AND)    # 2
            op(1, v.tensor_scalar, t1m[:, 0:8], t1[:, 0:8], 579, None, _MUL)       # 3
            op(3, v.tensor_tensor, s2[:, 0:8], t1m[:, 0:8], t0[:, 0:8], _ADD)      # 4
            op(4, v.tensor_scalar, j2[:, 0:8], s2[:, 0:8], 2047, None, _AND)       # 5
            op(5, v.tensor_copy, idx[:, 0:8], j2[:, 0:8])                          # 6
            # remainder of order-2 (cols 8:128)
            v.wait_ge(s_load[0], 16)
            v.wait_ge(s_load[1], 16)
            op(0, v.tensor_scalar, t1[:, 8:128], v64[:, 136:256], 2047, None, _AND)  # 7
            op(0, v.tensor_scalar, t0[:, 8:128], v64[:, 264:384], 2047, None, _AND)  # 8
            op(7, v.tensor_scalar, t1m[:, 8:128], t1[:, 8:128], 579, None, _MUL)     # 9
            op(9, v.tensor_tensor, s2[:, 8:128], t1m[:, 8:128], t0[:, 8:128], _ADD)  # 10
            op(10, v.tensor_scalar, j2[:, 8:128], s2[:, 8:128], 2047, None, _AND)    # 11
            op(11, v.tensor_copy, idx[:, 8:128], j2[:, 8:128])                       # 12
            # order-3 (full width; s2 complete after op 10)
            v.wait_ge(s_load[2], 16)
            op(0, v.tensor_scalar, t2[:, :], v64[:, 0:128], 2047, None, _AND)     # 13
            op(13, v.tensor_scalar, t2m[:, :], t2[:, :], 1417, None, _MUL)        # 14
            op(14, v.tensor_tensor, s3[:, :], t2m[:, :], s2[:, :], _ADD)          # 15
            op(15, v.tensor_scalar, j3[:, :], s3[:, :], 2047, None, _AND)         # 16
            op(16, v.tensor_copy, idx[:, 128:256], j3[:, :])                      # 17
            assert n == 17

        @block.gpsimd
        def _(gp):
            gp.load_library(mlp)
            # Pool-sequencer instruction dispatch costs ~0.4us apiece, so:
            # (a) stage the num_idxs registers ONCE, before the data-dependent
            # wait (to_reg per call emitted 16 MOVEs, ~5us of them on the
            # critical path before the first gather); (b) emit only the 3
            # distinct v_hash waits instead of one per call.
            nregs = {}
            for spc in sorted(set(CHUNK_SLOTS)):
                r = stack.enter_context(gp.register(f"nidx{spc}"))
                gp.reg_mov(r, 128 * spc)
                nregs[spc] = gp.snap(r)
            # one dma_gather tops out between 1024 and 2048 descriptors on HW
            # (SWDGE ring in-flight cap); finer chunks also start transfers
            # earlier and interleave the writeback DMAs with gather reads
            last_wait = 0
            for o in range(N_ORDERS):
                sl0 = 0
                for h, spc in enumerate(CHUNK_SLOTS):
                    # chunk-0 fast path ready at 6, rest of order-2 at 12,
                    # order-3 at 17
                    need = (6 if h == 0 else 12) if o == 0 else 17
                    if need > last_wait:
                        gp.wait_ge(v_hash, need)
                        last_wait = need
                    gp.dma_gather(
                        dst[o][:, sl0 : sl0 + spc, :],
                        tbl[NUM_BUCKETS * o : NUM_BUCKETS * (o + 1), :],
                        idx[:, 128 * o + 8 * sl0 : 128 * o + 8 * (sl0 + spc)],
                        128 * spc,
                        nregs[spc],
                        FUSED_DIM,
                        queue_num=GATHER_QUEUES[N_SPLIT * o + h],
                    ).then_inc(s_g[N_SPLIT * o + h], 16)
                    sl0 += spc

    nc.compile()
    return nc


_NC = None


def _get_nc():
    global _NC
    if _NC is None:
        _NC = _build_nc()
    return _NC


def _shard_tokens(tokens):
    """Per-core [128, 384] i32 tiles: col-block k in {0,1,2} holds the
    (2-k)-shifted token stream laid out so cell (p, m) is the token at
    position 16m + p%16 - (2-k) (0-padded past the row start), replicated
    across the 8 partition groups."""
    tokens = np.asarray(tokens, dtype=np.int64)
    maps = []
    q = np.arange(16)[:, None]  # partition % 16
    m = np.arange(128)[None, :]
    for c in range(N_CORES):
        b, s0 = c // 2, (c % 2) * POS
        halo = np.concatenate([np.zeros(2, np.int64), tokens[b]])  # halo[i] = t[i-2]
        blocks = [halo[s0 + 16 * m + q + k] for k in range(3)]  # [16, 128] each
        tile = np.concatenate(blocks, axis=1).astype(np.int32)  # [16, 384]
        # fast-path mini-block: duplicate the first 8 cols of blocks 1 and 2
        tile = np.concatenate([tile, tile[:, 128:136], tile[:, 256:264]], axis=1)
        maps.append(np.ascontiguousarray(np.tile(tile, (8, 1))))
    return maps


def _fuse_tables(tables):
    """[8, 2048, 256] -> [2*2048, 1024]: per-order head-concat with each head's
    table rotated by its additive hash constant, so all heads share one index."""
    tables = np.asarray(tables, dtype=np.float32).reshape(8, NUM_BUCKETS, 256)
    j = np.arange(NUM_BUCKETS)
    fused = np.empty((N_ORDERS, NUM_BUCKETS, 4, 256), np.float32)
    for h in range(4):
        fused[0, :, h, :] = tables[h][(j + _C2[h]) & (NUM_BUCKETS - 1)]
        fused[1, :, h, :] = tables[4 + h][(j + _C3[h]) & (NUM_BUCKETS - 1)]
    return np.ascontiguousarray(fused.reshape(N_ORDERS * NUM_BUCKETS, FUSED_DIM))


def _in_maps(tokens, tables):
    tbl = _fuse_tables(tables)
    return [{"tokens": tw, "tables": tbl} for tw in _shard_tokens(tokens)]


def kernel(tokens, tables):
    from concourse.bass_utils import run_bass_kernel_spmd

    res = run_bass_kernel_spmd(
        _get_nc(), _in_maps(tokens, tables), list(range(N_CORES))
    )
    parts = [np.asarray(res.results[c]["out"]) for c in range(N_CORES)]
    return np.concatenate(parts, axis=0).reshape(B, S, D)


def run_traced(tokens, tables, **kw):
    """Timing/profiling run; returns the full BassKernelResults."""
    from concourse.bass_utils import run_bass_kernel_spmd

    return run_bass_kernel_spmd(
        _get_nc(), _in_maps(tokens, tables), list(range(N_CORES)), trace=True, **kw
    )



# revision 76
# speedup vs baseline: 1.1476x; 1.0200x over previous
"""Trainium2 Bass kernel for EngramMemory embedding lookup.

Computation (matches the jax reference):
  tokens [4, 4096] int64, tables [8, 2048, 256] f32
  For each order o in (2, 3) and head h in 0..3, hash the o-gram ending at
  each position into a bucket (mod 2048) and gather the 256-wide embedding
  row; concatenate all 8 (order, head) chunks -> out [4, 4096, 2048] f32.

The hash recurrence  h = (h*P + tok + seed) % 2048  telescopes to an affine
form mod 2048 (P = 1000003 == 579, P^2 == 1417 mod 2048):
  order2 bucket_h = (x2 + C2[h]) & 2047,  x2 = 579*t[s-1] + t[s]
  order3 bucket_h = (x3 + C3[h]) & 2047,  x3 = 1417*t[s-2] + 579*t[s-1] + t[s]
(tokens pre-masked to 11 bits so products stay < 2^24, exact in the DVE's
fp32 multiply path; C_oh = seed_oh * (P+..+1) mod 2048).

Within an order, the four heads differ only by the additive constant C_oh.
Host-side we therefore build per-order fused tables
  F_o[j] = concat_h T_oh[(j + C_oh) & 2047]        -> [2048, 4*256] f32
so the device gathers ONE 4 KiB row per (position, order) with the shared
index j = x_o & 2047. This cuts SWDGE gather descriptors 4x (the Q7
descriptor-generation rate, ~8.5 ns/descriptor on one core pair, is the
serial bottleneck) and makes every gathered row land contiguously in the
output layout.

Sharding: data-parallel over the 16384 token positions, 2048 per core
(core c takes batch row c//2, half c%2); tables replicated. Per core:
  1. DMA shifted token streams [128, 400] i32 (8 KiB fast-path mini-block first) (host supplies them in the
     index-tile layout: cell (p, m) of block k is the token at position
     16m + p%16 - (2-k), replicated across the 8 partition groups).
  2. 17 DVE int ops -> index tile [128, 256] i16: cell (p, o*128 + m) =
     fused-row index for order o at position 16m + p%16. This matches the
     SWDGE gather index wrap (idx j at partition j%16, col j//16, i.e.
     j = position), is born replicated for all Q7 pairs, and makes the
     gathered rows land position-major.
  3. 16x dma_gather, 8 position-range chunks per order (one call tops out
     between 1024 and 2048 descriptors = the 128-entry/ring in-flight cap x
     16 rings; chunks also pipeline with the writeback; a 128-position
     first chunk rides the fast-path index slice), queues cycling over the
     4 Q7 core pairs.
  4. 16 HWDGE out-DMAs (3-dim APs spanning all 128 partitions -> all 16 DMA
     engines), alternating between the sync and scalar HWDGE engines since
     one engine's descriptor generation (~2.5 us/MiB) cannot feed the DMA
     bus alone.

Measured on TRN2 (8 cores): ~103 us HW exec (best; ambient shared-HBM
contention adds up to ~20% in bad phases), vs a ~89 us saturated-DMA-bus
floor (32 MiB/core: 16 MiB random 4 KiB gather reads + 16 MiB writeback at
~25 GB/s x 16 engines; the DMA engines measure 100% busy in the steady
phase). The ~19.5 us serial prefix is preamble + token load + hash + first
descriptor generation and ucode IRAM fetch.
"""

import os
import sys
from contextlib import ExitStack

import numpy as np

for _p in ("/opt/trn_rl_repo", os.path.expanduser("~/.axon_site/_ro/trn_rl_repo")):
    if os.path.isdir(_p) and _p not in sys.path:
        sys.path.insert(0, _p)

import concourse.bacc as bacc
import concourse.mybir as mybir

N_CORES = 8
B, S, D = 4, 4096, 2048
POS = (B * S) // N_CORES  # 2048 positions per core
NUM_BUCKETS = 2048
FUSED_DIM = 4 * 256  # one fused row: 4 heads x head_dim
N_ORDERS = 2

# Affine hash constants (see module docstring). seeds = 1337 + 97*o + 17*h.
_C2 = [((1337 + 97 * 2 + 17 * h) * 580) % 2048 for h in range(4)]
_C3 = [((1337 + 97 * 3 + 17 * h) * 1997) % 2048 for h in range(4)]

_AND = mybir.AluOpType.bitwise_and
_MUL = mybir.AluOpType.mult
_ADD = mybir.AluOpType.add

# gather chunk sizes per order, in 128-position slots (sum 16 = 2048
# positions). The tiny first chunk rides a fast-path index slice computed
# right after the first token block lands, pulling the first descriptor
# generation (and the gather ucode's IRAM fetch) several us earlier. queue
# (= Q7 core pair) cycles per call so desc-gen rings don't serialize.
CHUNK_SLOTS = [1, 2, 2, 2, 2, 2, 2, 3]
N_SPLIT = len(CHUNK_SLOTS)
GATHER_QUEUES = [i % 4 for i in range(2 * N_SPLIT)]


def _build_nc():
    from concourse.library_config import mlp

    nc = bacc.Bacc("TRN2", num_swdge_queues=max(GATHER_QUEUES) + 1)
    # tokens[p, k*128 + m] = token at position 16m + p%16 - (2-k)  (0-padded),
    # i.e. col-block k holds the (2-k)-shifted token stream in the layout the
    # index tile needs (idx j for position 16m+q lives at partition q, col m)
    # cols 384:400 duplicate the fast-path operand slices (cols 128:136 and
    # 256:264) so one 8 KiB DMA + one completion sem gates the first hash ops
    tok = nc.declare_dram_parameter("tokens", [128, 400], mybir.dt.int32, isOutput=False)
    tbl = nc.declare_dram_parameter(
        "tables", [N_ORDERS * NUM_BUCKETS, FUSED_DIM], mybir.dt.float32, isOutput=False
    )
    out = nc.declare_dram_parameter("out", [POS, D], mybir.dt.float32, isOutput=True)

    with (
        nc.Block() as block,
        ExitStack() as stack,
    ):
        sem = lambda name: stack.enter_context(nc.semaphore(name))  # noqa: E731
        sb = lambda name, shape, dt: stack.enter_context(nc.sbuf_tensor(name, shape, dt))  # noqa: E731

        s_load = [sem(f"s_load{k}") for k in range(3)]
        s_mini = sem("s_mini")
        v_hash = sem("v_hash")
        s_g = [sem(f"s_g{i}") for i in range(2 * N_SPLIT)]  # one per gather call (per queue)
        s_out = sem("s_out")
        s_out_b = sem("s_out_b")

        # All hash tiles live on all 128 partitions: the hash is computed
        # 8x redundantly (DVE cost is per-partition elements, so this is
        # free) and the index tile comes out already replicated for the
        # Q7 pairs -- no replication DMAs on the critical path.
        v64 = sb("v64", [128, 384], mybir.dt.int32)
        v64x = sb("v64x", [128, 16], mybir.dt.int32)
        t0 = sb("t0", [128, 128], mybir.dt.int32)
        t1 = sb("t1", [128, 128], mybir.dt.int32)
        t2 = sb("t2", [128, 128], mybir.dt.int32)
        t1m = sb("t1m", [128, 128], mybir.dt.int32)
        t2m = sb("t2m", [128, 128], mybir.dt.int32)
        s2 = sb("s2", [128, 128], mybir.dt.int32)
        s3 = sb("s3", [128, 128], mybir.dt.int32)
        j2 = sb("j2", [128, 128], mybir.dt.int32)
        j3 = sb("j3", [128, 128], mybir.dt.int32)
        idx = sb("idx", [128, N_ORDERS * 128], mybir.dt.int16)
        dst = [sb(f"dst{o}", [128, 16, FUSED_DIM], mybir.dt.float32) for o in range(N_ORDERS)]

        @block.sync
        def _(sync):
            # fast-path mini-block first, then the three col-block loads
            # ordered so the order-2 hash operands arrive before order-3's
            sync.dma_start(v64x[:, :], tok[:, 384:400]).then_inc(s_mini, 16)
            sync.dma_start(v64[:, 128:256], tok[:, 128:256]).then_inc(s_load[0], 16)
            sync.dma_start(v64[:, 256:384], tok[:, 256:384]).then_inc(s_load[1], 16)
            sync.dma_start(v64[:, 0:128], tok[:, 0:128]).then_inc(s_load[2], 16)
            # Gather rows land pos-major: row for position 128*sl + p sits at
            # (partition p, slot sl). Each out-DMA is a clean 3-dim AP over
            # all 128 partitions -> all 16 DMA engines. HWDGE descriptor
            # generation runs ~2.5us/MiB per engine -- barely above the DMA
            # bus rate -- so the writeback DMAs alternate between the two
            # HWDGE engines (sync here, scalar below) to keep the bus fed.
            ov = out.rearrange("(sl p) (o e) -> o p sl e", sl=16, p=128, e=FUSED_DIM)
            n_sync = 0
            for o in range(N_ORDERS):
                sl0 = 0
                for h, spc in enumerate(CHUNK_SLOTS):
                    if (N_SPLIT * o + h) % 2 == 0:
                        sync.wait_ge(s_g[N_SPLIT * o + h], 16)
                        sync.dma_start(
                            ov[o][:, sl0 : sl0 + spc, :],
                            dst[o][:, sl0 : sl0 + spc, :],
                        ).then_inc(s_out, 16)
                        n_sync += 1
                    sl0 += spc
            sync.wait_ge(s_out, 16 * n_sync)
            sync.wait_ge(s_out_b, 16 * (N_SPLIT * N_ORDERS - n_sync))

        @block.scalar
        def _(sc):
            ovb = out.rearrange("(sl p) (o e) -> o p sl e", sl=16, p=128, e=FUSED_DIM)
            for o in range(N_ORDERS):
                sl0 = 0
                for h, spc in enumerate(CHUNK_SLOTS):
                    if (N_SPLIT * o + h) % 2 == 1:
                        sc.wait_ge(s_g[N_SPLIT * o + h], 16)
                        sc.dma_start(
                            ovb[o][:, sl0 : sl0 + spc, :],
                            dst[o][:, sl0 : sl0 + spc, :],
                        ).then_inc(s_out_b, 16)
                    sl0 += spc

        @block.vector
        def _(v):
            # Every DVE op bumps v_hash; DVE has no same-engine interlocks, so
            # dependent ops wait on the cumulative count of their producers.
            # Order-2 path first: its index tile gates the first gathers.
            n = 0

            def op(wait, fn, *args):
                nonlocal n
                if wait:
                    v.wait_ge(v_hash, wait)
                fn(*args).then_inc(v_hash, 1)
                n += 1

            # fast path: idx cols 0:8 (gather chunk 0, 128 positions) from the
            # mini-block, so the first desc-gen starts several us earlier
            v.wait_ge(s_mini, 16)
            op(0, v.tensor_scalar, t1[:, 0:8], v64x[:, 0:8], 2047, None, _AND)     # 1
            op(0, v.tensor_scalar, t0[:, 0:8], v64x[:, 8:16], 2047, None, _AND)    # 2
            op(1, v.tensor_scalar, t1m[:, 0:8], t1[:, 0:8], 579, None, _MUL)       # 3
            op(3, v.tensor_tensor, s2[:, 0:8], t1m[:, 0:8], t0[:, 0:8], _ADD)      # 4
            op(4, v.tensor_scalar, j2[:, 0:8], s2[:, 0:8], 2047, None, _AND)       # 5
            op(5, v.tensor_copy, idx[:, 0:8], j2[:, 0:8])                          # 6
            # remainder of order-2 (cols 8:128)
            v.wait_ge(s_load[0], 16)
            v.wait_ge(s_load[1], 16)
            op(0, v.tensor_scalar, t1[:, 8:128], v64[:, 136:256], 2047, None, _AND)  # 7
            op(0, v.tensor_scalar, t0[:, 8:128], v64[:, 264:384], 2047, None, _AND)  # 8
            op(7, v.tensor_scalar, t1m[:, 8:128], t1[:, 8:128], 579, None, _MUL)     # 9
            op(9, v.tensor_tensor, s2[:, 8:128], t1m[:, 8:128], t0[:, 8:128], _ADD)  # 10
            op(10, v.tensor_scalar, j2[:, 8:128], s2[:, 8:128], 2047, None, _AND)    # 11
            op(11, v.tensor_copy, idx[:, 8:128], j2[:, 8:128])                       # 12
            # order-3 (full width; s2 complete after op 10)
            v.wait_ge(s_load[2], 16)
            op(0, v.tensor_scalar, t2[:, :], v64[:, 0:128], 2047, None, _AND)     # 13
            op(13, v.tensor_scalar, t2m[:, :], t2[:, :], 1417, None, _MUL)        # 14
            op(14, v.tensor_tensor, s3[:, :], t2m[:, :], s2[:, :], _ADD)          # 15
            op(15, v.tensor_scalar, j3[:, :], s3[:, :], 2047, None, _AND)         # 16
            op(16, v.tensor_copy, idx[:, 128:256], j3[:, :])                      # 17
            assert n == 17

        @block.gpsimd
        def _(gp):
            gp.load_library(mlp)
            # Pool-sequencer instruction dispatch costs ~0.4us apiece, so:
            # (a) stage the num_idxs registers ONCE, before the data-dependent
            # wait (to_reg per call emitted 16 MOVEs, ~5us of them on the
            # critical path before the first gather); (b) emit only the 3
            # distinct v_hash waits instead of one per call.
            nregs = {}
            for spc in sorted(set(CHUNK_SLOTS)):
                r = stack.enter_context(gp.register(f"nidx{spc}"))
                gp.reg_mov(r, 128 * spc)
                nregs[spc] = gp.snap(r)
            # one dma_gather tops out between 1024 and 2048 descriptors on HW
            # (SWDGE ring in-flight cap); finer chunks also start transfers
            # earlier and interleave the writeback DMAs with gather reads
            last_wait = 0
            for o in range(N_ORDERS):
                sl0 = 0
                for h, spc in enumerate(CHUNK_SLOTS):
                    # chunk-0 fast path ready at 6, rest of order-2 at 12,
                    # order-3 at 17
                    need = (6 if h == 0 else 12) if o == 0 else 17
                    if need > last_wait:
                        gp.wait_ge(v_hash, need)
                        last_wait = need
                    gp.dma_gather(
                        dst[o][:, sl0 : sl0 + spc, :],
                        tbl[NUM_BUCKETS * o : NUM_BUCKETS * (o + 1), :],
                        idx[:, 128 * o + 8 * sl0 : 128 * o + 8 * (sl0 + spc)],
                        128 * spc,
                        nregs[spc],
                        FUSED_DIM,
                        queue_num=GATHER_QUEUES[N_SPLIT * o + h],
                    ).then_inc(s_g[N_SPLIT * o + h], 16)
                    sl0 += spc

    nc.compile()
    return nc


_NC = None


def _get_nc():
    global _NC
    if _NC is None:
        _NC = _build_nc()
    return _NC


def _shard_tokens(tokens):
    """Per-core [128, 384] i32 tiles: col-block k in {0,1,2} holds the
    (2-k)-shifted token stream laid out so cell (p, m) is the token at
    position 16m + p%16 - (2-k) (0-padded past the row start), replicated
    across the 8 partition groups."""
    tokens = np.asarray(tokens, dtype=np.int64)
    maps = []
    q = np.arange(16)[:, None]  # partition % 16
    m = np.arange(128)[None, :]
    for c in range(N_CORES):
        b, s0 = c // 2, (c % 2) * POS
        halo = np.concatenate([np.zeros(2, np.int64), tokens[b]])  # halo[i] = t[i-2]
        blocks = [halo[s0 + 16 * m + q + k] for k in range(3)]  # [16, 128] each
        tile = np.concatenate(blocks, axis=1).astype(np.int32)  # [16, 384]
        # fast-path mini-block: duplicate the first 8 cols of blocks 1 and 2
        tile = np.concatenate([tile, tile[:, 128:136], tile[:, 256:264]], axis=1)
        maps.append(np.ascontiguousarray(np.tile(tile, (8, 1))))
    return maps


def _fuse_tables(tables):
    """[8, 2048, 256] -> [2*2048, 1024]: per-order head-concat with each head's
    table rotated by its additive hash constant, so all heads share one index."""
    tables = np.asarray(tables, dtype=np.float32).reshape(8, NUM_BUCKETS, 256)
    j = np.arange(NUM_BUCKETS)
    fused = np.empty((N_ORDERS, NUM_BUCKETS, 4, 256), np.float32)
    for h in range(4):
        fused[0, :, h, :] = tables[h][(j + _C2[h]) & (NUM_BUCKETS - 1)]
        fused[1, :, h, :] = tables[4 + h][(j + _C3[h]) & (NUM_BUCKETS - 1)]
    return np.ascontiguousarray(fused.reshape(N_ORDERS * NUM_BUCKETS, FUSED_DIM))


def _in_maps(tokens, tables):
    tbl = _fuse_tables(tables)
    return [{"tokens": tw, "tables": tbl} for tw in _shard_tokens(tokens)]


def kernel(tokens, tables):
    from concourse.bass_utils import run_bass_kernel_spmd

    res = run_bass_kernel_spmd(
        _get_nc(), _in_maps(tokens, tables), list(range(N_CORES))
    )
    parts = [np.asarray(res.results[c]["out"]) for c in range(N_CORES)]
    return np.concatenate(parts, axis=0).reshape(B, S, D)


def run_traced(tokens, tables, **kw):
    """Timing/profiling run; returns the full BassKernelResults."""
    from concourse.bass_utils import run_bass_kernel_spmd

    return run_bass_kernel_spmd(
        _get_nc(), _in_maps(tokens, tables), list(range(N_CORES)), trace=True, **kw
    )

